# revision 10
# baseline (speedup 1.0000x reference)
"""Trainium2 Bass kernel for nn_CrossAttention (FFT-query cross attention).

Math:
  out = softmax((Re(FFT(query, axis=1)) @ Wq^T + bq) @ (key @ Wk^T + bk)^T / sqrt(D)) @ key

Key identities used:
  * Re(FFT(x))[j] = sum_n x[n] cos(2*pi*j*n/N): a matmul with a cosine matrix.
  * cos rows satisfy C[N-j] = C[j]  =>  q rows mirror:  q[j] == q[N-j].
    The whole downstream pipeline is row-wise in q, so out[b, j] == out[b, N-j].
    Only rows j = 0..1024 are computed on device (padded to 1152 = 9*128);
    rows 1025..2047 are mirrored from rows 1023..1 on the host.
  * cos cols satisfy C[:, n] = C[:, N-n]  =>  fold x into
    y[0] = x[0], y[n] = x[n] + x[N-n] (n=1..1023), y[1024] = x[1024]
    and contract over only 1025 terms (plus one ones-row for the bq bias).
  * bk drops out of softmax entirely (adds a per-query-row constant to scores).
  * The 1/sqrt(D) scale is folded into the cosine table.
  * 1/rowsum of softmax is applied to the final [128, 256] output tiles, not
    to the [128, 2048] probability tiles.

Per-core layout (core b handles batch b; 8 cores, 8 batches):
  MM-A: z[n, d]   = y @ Wq^T            lhsT = y^T (host),   rhs = Wq^T (host)
  MM-C: kT[d, nk] = Wk @ key^T          lhsT = Wk^T (host),  rhs = key^T (host)
  MM-B: qsT[d, j] = z^T @ (C/16)        lhsT = z,            rhs = cos table (host)
  MM-D: S[j, nk]  = qs @ k^T            lhsT = qsT,          rhs = kT
  softmax rows of S (two 1024-wide halves; exp via ACT with accum_out, P bf16)
  MM-T: P^T tiles via PE transpose (bf16)
  MM-E: o[j, d]   = P @ key             lhsT = P^T (bf16),   rhs = key bf16

Perf notes:
  * Everything scores-side is fp16 (11-bit mantissa, same precision class as
    the PE's f32r mode, but half the DMA bytes and FWL-capable weight loads).
  * P / value side is bf16: bf16 keeps fp32's exponent range, so tiny softmax
    tail probabilities don't flush to zero the way fp16 denormals would.
  * Matmul accumulation chains are interleaved across PSUM banks: consecutive
    PE instructions always target different banks so the drain of one overlaps
    the fill of the next (same-bank accumulation steps serialize).
"""

import numpy as np
import ml_dtypes

import concourse.bass as bass
import concourse.tile as tile
from concourse import bacc, mybir
from concourse.bass_utils import run_bass_kernel_spmd

B = 8
NSEQ = 2048          # query/key sequence length
D = 256              # feature dim
NQH = 1152           # computed query rows (9 tiles of 128; rows >1024 unused)
NFOLD = 1026         # folded contraction: 1025 cosine rows + 1 bias row
NJT = NQH // 128     # 9 query-row tiles
NKT = NSEQ // 128    # 16 key tiles
SCALE = 1.0 / 16.0   # 1/sqrt(D)

f32 = mybir.dt.float32
f32r = mybir.dt.float32r
bf16 = mybir.dt.bfloat16
fp16 = mybir.dt.float16

_compiled = {}


def _build_module():
    nc = bacc.Bacc("TRN2", target_bir_lowering=False, debug=False, num_devices=B)

    dram = {}
    def din(name, shape, dt=f32):
        dram[name] = nc.dram_tensor(name, list(shape), dt, kind="ExternalInput").ap()
    def dout(name, shape):
        dram[name] = nc.dram_tensor(name, list(shape), f32, kind="ExternalOutput").ap()

    din("yt", (D, 1025))          # folded query, transposed
    din("bq", (1, D))
    din("wqt", (D, D))            # Wq^T
    din("wkt", (D, D))            # Wk^T
    din("keyt", (D, NSEQ))        # key^T
    din("keyn", (NSEQ, D), bf16)  # key natural, bf16 (value side)
    din("cth", (NFOLD, NQH), bf16)  # cosine table hi (bf16)
    din("ctl", (NFOLD, NQH), bf16)  # cosine table lo (bf16 residual)
    din("ident", (128, 128), bf16)
    dout("ob", (NQH, D))

    with tile.TileContext(nc) as tc:
        _emit(nc, tc, dram)
    nc.compile()
    return nc


def _emit(nc, tc, dram):
    from contextlib import ExitStack

    with ExitStack() as ctx:
        const = ctx.enter_context(tc.tile_pool(name="const", bufs=1))
        zpool = ctx.enter_context(tc.tile_pool(name="z", bufs=1))
        qkpool = ctx.enter_context(tc.tile_pool(name="qk", bufs=1))

        # ---- constant loads, in phase-consumption order (A, C, B, loop) ----
        yt = [const.tile([128, 1025], f32r, tag=f"yt{i}", name=f"yt{i}") for i in range(2)]
        wqt = [const.tile([128, D], f32r, tag=f"wqt{i}", name=f"wqt{i}") for i in range(2)]
        for i in range(2):
            eng = nc.sync if i == 0 else nc.gpsimd
            eng.dma_start(yt[i][:], dram["yt"][i * 128:(i + 1) * 128, :].bitcast(f32r))
            eng.dma_start(wqt[i][:], dram["wqt"][i * 128:(i + 1) * 128, :].bitcast(f32r))
        wkt = [const.tile([128, D], f32r, tag=f"wkt{i}", name=f"wkt{i}") for i in range(2)]
        keyt = [const.tile([128, NSEQ], f32r, tag=f"keyt{i}", name=f"keyt{i}") for i in range(2)]
        for i in range(2):
            eng = nc.sync if i == 0 else nc.gpsimd
            eng.dma_start(wkt[i][:], dram["wkt"][i * 128:(i + 1) * 128, :].bitcast(f32r))
            eng.dma_start(keyt[i][:], dram["keyt"][i * 128:(i + 1) * 128, :].bitcast(f32r))
        cts = []
        for i in range(9):
            r = 128 if i < 8 else 2
            th = const.tile([r, NQH], bf16, tag=f"cth{i}", name=f"cth{i}")
            tl = const.tile([r, NQH], bf16, tag=f"ctl{i}", name=f"ctl{i}")
            nc.sync.dma_start(th[:], dram["cth"][i * 128:i * 128 + r, :])
            nc.gpsimd.dma_start(tl[:], dram["ctl"][i * 128:i * 128 + r, :])
            t = const.tile([r, NQH], f32r, tag=f"ct{i}", name=f"ct{i}")
            nc.vector.tensor_add(t[:], th[:], tl[:])
            cts.append(t)
        keyn = [const.tile([128, D], bf16, tag=f"keyn{i}", name=f"keyn{i}") for i in range(NKT)]
        for i in range(NKT):
            eng = nc.sync if i % 2 == 0 else nc.gpsimd
            eng.dma_start(keyn[i][:], dram["keyn"][i * 128:(i + 1) * 128, :])
        id_b = const.tile([128, 128], bf16, tag="ident", name="ident")
        nc.gpsimd.dma_start(id_b[:], dram["ident"][:])

        # ---- phase A: z = y @ Wq^T (9 row tiles; chains interleaved 4-5 wide)
        zbuf = []
        for i in range(8):
            zbuf.append(zpool.tile([128, D], f32r, tag=f"z{i}", name=f"z{i}"))
        zbuf.append(zpool.tile([2, D], f32r, tag="z8", name="z8"))  # row0: z[1024], row1: bq
        nc.sync.dma_start(zbuf[8][1:2, :], dram["bq"][:].bitcast(f32r))

        with tc.tile_pool(name="psA", bufs=5, space="PSUM") as psA:
            for grp in (range(0, 5), range(5, 9)):
                pss = {}
                for nt in grp:
                    pss[nt] = psA.tile([128, D], f32, tag="psA", name="psA")
                for kd in range(2):
                    for nt in grp:
                        m = 128 if nt < 8 else 1
                        nc.tensor.matmul(
                            pss[nt][:m, :], yt[kd][:, nt * 128:nt * 128 + m],
                            wqt[kd][:], start=(kd == 0), stop=(kd == 1))
                for nt in grp:
                    m = 128 if nt < 8 else 1
                    nc.vector.tensor_copy(zbuf[nt][:m, :], pss[nt][:m, :])

        # ---- phase C: kT = Wk @ key^T  [256, 2048]; 8 chains interleaved ----
        kT = [qkpool.tile([128, NSEQ], f32r, tag=f"kT{i}", name=f"kT{i}") for i in range(2)]
        with tc.tile_pool(name="psC", bufs=8, space="PSUM") as psC:
            pss = {}
            for dt in range(2):
                for c in range(4):
                    pss[(dt, c)] = psC.tile([128, 512], f32, tag="psC", name="psC")
            for kd in range(2):
                for dt in range(2):
                    for c in range(4):
                        sl = slice(c * 512, (c + 1) * 512)
                        nc.tensor.matmul(
                            pss[(dt, c)][:], wkt[kd][:, dt * 128:(dt + 1) * 128],
                            keyt[kd][:, sl], start=(kd == 0), stop=(kd == 1))
            for dt in range(2):
                for c in range(4):
                    sl = slice(c * 512, (c + 1) * 512)
                    nc.vector.tensor_copy(kT[dt][:, sl], pss[(dt, c)][:])

        # ---- phase B: qsT = z^T @ (C/16)  [256, 1152]; 6 chains interleaved --
        qsT = [qkpool.tile([128, NQH], f32r, tag=f"qsT{i}", name=f"qsT{i}") for i in range(2)]
        with tc.tile_pool(name="psB", bufs=6, space="PSUM") as psB:
            pss = {}
            for dt in range(2):
                for c in range(3):
                    pss[(dt, c)] = psB.tile([128, 384], f32, tag="psB", name="psB")
            for kt in range(9):
                kr = 128 if kt < 8 else 2
                for dt in range(2):
                    for c in range(3):
                        sl = slice(c * 384, (c + 1) * 384)
                        nc.tensor.matmul(
                            pss[(dt, c)][:], zbuf[kt][:kr, dt * 128:(dt + 1) * 128],
                            cts[kt][:kr, sl], start=(kt == 0), stop=(kt == 8))
            for dt in range(2):
                for c in range(3):
                    sl = slice(c * 384, (c + 1) * 384)
                    nc.vector.tensor_copy(qsT[dt][:, sl], pss[(dt, c)][:])

        # ---- phase D: attention over 9 query tiles, software-pipelined ----
        with ExitStack() as jctx:
            psS = jctx.enter_context(tc.tile_pool(name="psS", bufs=2, space="PSUM"))
            psT = jctx.enter_context(tc.tile_pool(name="psT", bufs=2, space="PSUM"))
            psO = jctx.enter_context(tc.tile_pool(name="psO", bufs=2, space="PSUM"))
            work = jctx.enter_context(tc.tile_pool(name="work", bufs=3))
            ptp = jctx.enter_context(tc.tile_pool(name="ptp", bufs=4))
            stats = jctx.enter_context(tc.tile_pool(name="stats", bufs=4))

            state = {}  # per-jt carried tiles
            for step in range(NJT + 2):
                if step < NJT:
                    jt = step
                    jsl = slice(jt * 128, (jt + 1) * 128)
                    # scores in two 1024-wide halves (2 psum banks each);
                    # within a half the two 512-chunks interleave the K steps
                    halves = []
                    for h in range(2):
                        sh = psS.tile([128, 1024], f32, tag="psS", name="psS")
                        for dt in range(2):
                            for c in range(2):
                                sl = slice(c * 512, (c + 1) * 512)
                                ksl = slice(h * 1024 + c * 512, h * 1024 + (c + 1) * 512)
                                nc.tensor.matmul(
                                    sh[:, sl], qsT[dt][:, jsl], kT[dt][:, ksl],
                                    start=(dt == 0), stop=(dt == 1))
                        halves.append(sh)
                    mx = [stats.tile([128, 1], f32, tag=f"mx{h}", name=f"mx{h}") for h in range(2)]
                    for h in range(2):
                        nc.vector.reduce_max(out=mx[h][:], in_=halves[h][:],
                                             axis=mybir.AxisListType.X, negate=True)
                    negmax = stats.tile([128, 1], f32, tag="negmax", name="negmax")
                    nc.vector.tensor_scalar_min(negmax[:], mx[0][:], mx[1][:])
                    p_t = work.tile([128, NSEQ], bf16, tag="p", name="p")
                    sm = [stats.tile([128, 1], f32, tag=f"sm{h}", name=f"sm{h}") for h in range(2)]
                    for h in range(2):
                        nc.scalar.activation(
                            out=p_t[:, h * 1024:(h + 1) * 1024], in_=halves[h][:],
                            func=mybir.ActivationFunctionType.Exp,
                            bias=negmax[:], scale=1.0, accum_out=sm[h][:])
                    rsum = stats.tile([128, 1], f32, tag="rsum", name="rsum")
                    nc.vector.tensor_scalar_add(rsum[:], sm[0][:], sm[1][:])
                    recip = stats.tile([128, 1], f32, tag="recip", name="recip")
                    nc.vector.reciprocal(recip[:], rsum[:])
                    state[jt] = (p_t, recip)

                if step >= 2:
                    jt = step - 2
                    p_t, recip = state.pop(jt)
                    # 16 transposes packed 4-per-psum-bank, then one DVE copy
                    # per bank, then the 16 E accumulation steps (2 chains)
                    pt_sbs = []
                    for g in range(4):
                        pt_ps = psT.tile([128, 512], bf16, tag="psT", name="psT",
                                         padded_shape=[128, 1024])
                        for q in range(4):
                            kt = g * 4 + q
                            nc.tensor.matmul(pt_ps[:, q * 128:(q + 1) * 128],
                                             p_t[:, kt * 128:(kt + 1) * 128],
                                             id_b[:], is_transpose=True,
                                             start=True, stop=True)
                        pt_sb = ptp.tile([128, 512], bf16, tag="pt", name="pt")
                        nc.vector.tensor_copy(pt_sb[:], pt_ps[:])
                        pt_sbs.append(pt_sb)
                    po = [psO.tile([128, D], f32, tag="psO", name="psO",
                                   padded_shape=[128, 512])
                          for _ in range(2)]
                    for kt in range(NKT):
                        g, q = divmod(kt, 4)
                        nc.tensor.matmul(po[kt % 2][:],
                                         pt_sbs[g][:, q * 128:(q + 1) * 128],
                                         keyn[kt][:],
                                         start=(kt < 2), stop=(kt >= NKT - 2))
                    osb0 = work.tile([128, D], f32, tag="osb0", name="osb0")
                    nc.vector.tensor_scalar_mul(osb0[:], po[0][:], recip[:])
                    osb = work.tile([128, D], f32, tag="osb", name="osb")
                    nc.vector.scalar_tensor_tensor(
                        out=osb[:], in0=po[1][:], scalar=recip[:], in1=osb0[:],
                        op0=mybir.AluOpType.mult, op1=mybir.AluOpType.add)
                    nc.sync.dma_start(dram["ob"][jt * 128:(jt + 1) * 128, :], osb[:])


def _host_prep(query, key, Wq, bq, Wk, bk):
    """Build per-core input maps (fold+transpose query, transpose key/weights,
    cosine table)."""
    query = np.ascontiguousarray(query, dtype=np.float32)
    key = np.ascontiguousarray(key, dtype=np.float32)

    nn = np.arange(NFOLD - 1, dtype=np.float64)          # 0..1024
    jj = np.arange(NQH, dtype=np.float64)
    ct = np.empty((NFOLD, NQH), dtype=np.float32)
    ct[:-1] = (np.cos(2.0 * np.pi * np.outer(nn, jj) / NSEQ) * SCALE).astype(np.float32)
    ct[-1] = SCALE  # bias row (ones * scale)
    cth = ct.astype(ml_dtypes.bfloat16)
    ctl = (ct - cth.astype(np.float32)).astype(ml_dtypes.bfloat16)

    wqt = np.ascontiguousarray(Wq.T, dtype=np.float32)
    wkt = np.ascontiguousarray(Wk.T, dtype=np.float32)
    bq2 = np.ascontiguousarray(bq.reshape(1, D), dtype=np.float32)

    in_maps = []
    for b in range(B):
        x = query[b]
        y = np.empty((1025, D), dtype=np.float32)
        y[0] = x[0]
        y[1:1024] = x[1:1024] + x[2047:1024:-1]
        y[1024] = x[1024]
        in_maps.append({
            "yt": np.ascontiguousarray(y.T),
            "bq": bq2,
            "wqt": wqt,
            "wkt": wkt,
            "keyt": np.ascontiguousarray(key[b].T),
            "keyn": np.ascontiguousarray(key[b]).astype(ml_dtypes.bfloat16),
            "cth": cth,
            "ctl": ctl,
            "ident": np.eye(128, dtype=ml_dtypes.bfloat16),
        })
    return in_maps


def kernel(query, key, Wq, bq, Wk, bk, _trace=False, _trace_kwargs=None):
    if "nc" not in _compiled:
        _compiled["nc"] = _build_module()
    nc = _compiled["nc"]

    in_maps = _host_prep(query, key, Wq, bq, Wk, bk)
    kw = {}
    if _trace:
        kw["trace"] = True
        if _trace_kwargs:
            kw.update(_trace_kwargs)
    res = run_bass_kernel_spmd(nc, in_maps, core_ids=list(range(B)), **kw)
    _compiled["last_results"] = res

    out = np.empty((B, NSEQ, D), dtype=np.float32)
    for b in range(B):
        ob = res.results[b]["ob"]
        out[b, :1025] = ob[:1025]
        out[b, 1025:] = ob[1023:0:-1]
    return out


# revision 11
# speedup vs baseline: 1.1117x; 1.1117x over previous
"""Trainium2 Bass kernel for nn_CrossAttention (FFT-query cross attention).

Math:
  out = softmax((Re(FFT(query, axis=1)) @ Wq^T + bq) @ (key @ Wk^T + bk)^T / sqrt(D)) @ key

Key identities used:
  * Re(FFT(x))[j] = sum_n x[n] cos(2*pi*j*n/N): a matmul with a cosine matrix.
  * cos rows satisfy C[N-j] = C[j]  =>  q rows mirror:  q[j] == q[N-j].
    The whole downstream pipeline is row-wise in q, so out[b, j] == out[b, N-j].
    Only rows j = 0..1024 are computed on device (padded to 1152 = 9*128);
    rows 1025..2047 are mirrored from rows 1023..1 on the host.
  * cos cols satisfy C[:, n] = C[:, N-n]  =>  fold x into
    y[0] = x[0], y[n] = x[n] + x[N-n] (n=1..1023), y[1024] = x[1024]
    and contract over only 1025 terms (plus one ones-row for the bq bias).
  * bk drops out of softmax entirely (adds a per-query-row constant to scores).
  * The 1/sqrt(D) scale is folded into the cosine table.
  * 1/rowsum of softmax is applied to the final [128, 256] output tiles, not
    to the [128, 2048] probability tiles.

Per-core layout (core b handles batch b; 8 cores, 8 batches):
  MM-A: z[n, d]   = y @ Wq^T            lhsT = y^T (host),   rhs = Wq^T (host)
  MM-C: kT[d, nk] = Wk @ key^T          lhsT = Wk^T (host),  rhs = key^T (host)
  MM-B: qsT[d, j] = z^T @ (C/16)        lhsT = z,            rhs = cos table (host)
  MM-D: S[j, nk]  = qs @ k^T            lhsT = qsT,          rhs = kT
  softmax rows of S (two 1024-wide halves; exp via ACT with accum_out, P bf16)
  MM-T: P^T tiles via PE transpose (bf16)
  MM-E: o[j, d]   = P @ key             lhsT = P^T (bf16),   rhs = key bf16

Perf notes:
  * Everything scores-side is fp16 (11-bit mantissa, same precision class as
    the PE's f32r mode, but half the DMA bytes and FWL-capable weight loads).
  * P / value side is bf16: bf16 keeps fp32's exponent range, so tiny softmax
    tail probabilities don't flush to zero the way fp16 denormals would.
  * Matmul accumulation chains are interleaved across PSUM banks: consecutive
    PE instructions always target different banks so the drain of one overlaps
    the fill of the next (same-bank accumulation steps serialize).
"""

import numpy as np
import ml_dtypes

import concourse.bass as bass
import concourse.tile as tile
from concourse import bacc, mybir
from concourse.bass_utils import run_bass_kernel_spmd

B = 8
NSEQ = 2048          # query/key sequence length
D = 256              # feature dim
NQH = 1152           # computed query rows (9 tiles of 128; rows >1024 unused)
NFOLD = 1026         # folded contraction: 1025 cosine rows + 1 bias row
NJT = NQH // 128     # 9 query-row tiles
NKT = NSEQ // 128    # 16 key tiles
SCALE = 1.0 / 16.0   # 1/sqrt(D)

f32 = mybir.dt.float32
f32r = mybir.dt.float32r
bf16 = mybir.dt.bfloat16
fp16 = mybir.dt.float16

_compiled = {}


def _build_module():
    nc = bacc.Bacc("TRN2", target_bir_lowering=False, debug=False, num_devices=B)

    dram = {}
    def din(name, shape, dt=f32):
        dram[name] = nc.dram_tensor(name, list(shape), dt, kind="ExternalInput").ap()
    def dout(name, shape):
        dram[name] = nc.dram_tensor(name, list(shape), f32, kind="ExternalOutput").ap()

    din("yt", (D, 1025))          # folded query, transposed
    din("bq", (1, D))
    din("wqt", (D, D))            # Wq^T
    din("wkt", (D, D))            # Wk^T
    din("keyt", (D, NSEQ))        # key^T
    din("keyn", (NSEQ, D), bf16)  # key natural, bf16 (value side)
    din("cth", (NFOLD, NQH), bf16)  # cosine table hi (bf16)
    din("ctl", (NFOLD, NQH), bf16)  # cosine table lo (bf16 residual)
    din("ident", (128, 128), bf16)
    dout("ob", (NQH, D))

    with tile.TileContext(nc) as tc:
        _emit(nc, tc, dram)
    nc.compile()
    return nc


def _emit(nc, tc, dram):
    from contextlib import ExitStack

    with ExitStack() as ctx:
        const = ctx.enter_context(tc.tile_pool(name="const", bufs=1))
        zpool = ctx.enter_context(tc.tile_pool(name="z", bufs=1))
        qkpool = ctx.enter_context(tc.tile_pool(name="qk", bufs=1))

        # ---- constant loads, in phase-consumption order (A, C, B, loop) ----
        yt = [const.tile([128, 1025], f32r, tag=f"yt{i}", name=f"yt{i}") for i in range(2)]
        wqt = [const.tile([128, D], f32r, tag=f"wqt{i}", name=f"wqt{i}") for i in range(2)]
        for i in range(2):
            nc.sync.dma_start(yt[i][:], dram["yt"][i * 128:(i + 1) * 128, :].bitcast(f32r))
            nc.sync.dma_start(wqt[i][:], dram["wqt"][i * 128:(i + 1) * 128, :].bitcast(f32r))
        wkt = [const.tile([128, D], f32r, tag=f"wkt{i}", name=f"wkt{i}") for i in range(2)]
        keyt = [const.tile([128, NSEQ], f32r, tag=f"keyt{i}", name=f"keyt{i}") for i in range(2)]
        for i in range(2):
            nc.sync.dma_start(wkt[i][:], dram["wkt"][i * 128:(i + 1) * 128, :].bitcast(f32r))
            nc.sync.dma_start(keyt[i][:], dram["keyt"][i * 128:(i + 1) * 128, :].bitcast(f32r))
        cts = []
        for i in range(9):
            r = 128 if i < 8 else 2
            th = const.tile([r, NQH], bf16, tag=f"cth{i}", name=f"cth{i}")
            tl = const.tile([r, NQH], bf16, tag=f"ctl{i}", name=f"ctl{i}")
            nc.sync.dma_start(th[:], dram["cth"][i * 128:i * 128 + r, :])
            nc.sync.dma_start(tl[:], dram["ctl"][i * 128:i * 128 + r, :])
            t = const.tile([r, NQH], f32r, tag=f"ct{i}", name=f"ct{i}")
            nc.vector.tensor_add(t[:], th[:], tl[:])
            cts.append(t)
        keyn = [const.tile([128, D], bf16, tag=f"keyn{i}", name=f"keyn{i}") for i in range(NKT)]
        for i in range(NKT):
            nc.sync.dma_start(keyn[i][:], dram["keyn"][i * 128:(i + 1) * 128, :])
        id_b = const.tile([128, 128], bf16, tag="ident", name="ident")
        nc.sync.dma_start(id_b[:], dram["ident"][:])

        # ---- phase A: z = y @ Wq^T (9 row tiles; chains interleaved 4-5 wide)
        zbuf = []
        for i in range(8):
            zbuf.append(zpool.tile([128, D], f32r, tag=f"z{i}", name=f"z{i}"))
        zbuf.append(zpool.tile([2, D], f32r, tag="z8", name="z8"))  # row0: z[1024], row1: bq
        nc.sync.dma_start(zbuf[8][1:2, :], dram["bq"][:].bitcast(f32r))

        with tc.tile_pool(name="psA", bufs=5, space="PSUM") as psA:
            for grp in (range(0, 5), range(5, 9)):
                pss = {}
                for nt in grp:
                    pss[nt] = psA.tile([128, D], f32, tag="psA", name="psA")
                for kd in range(2):
                    for nt in grp:
                        m = 128 if nt < 8 else 1
                        nc.tensor.matmul(
                            pss[nt][:m, :], yt[kd][:, nt * 128:nt * 128 + m],
                            wqt[kd][:], start=(kd == 0), stop=(kd == 1))
                for nt in grp:
                    m = 128 if nt < 8 else 1
                    nc.vector.tensor_copy(zbuf[nt][:m, :], pss[nt][:m, :])

        # ---- phase C: kT = Wk @ key^T  [256, 2048]; 8 chains interleaved ----
        kT = [qkpool.tile([128, NSEQ], f32r, tag=f"kT{i}", name=f"kT{i}") for i in range(2)]
        with tc.tile_pool(name="psC", bufs=8, space="PSUM") as psC:
            pss = {}
            for dt in range(2):
                for c in range(4):
                    pss[(dt, c)] = psC.tile([128, 512], f32, tag="psC", name="psC")
            for kd in range(2):
                for dt in range(2):
                    for c in range(4):
                        sl = slice(c * 512, (c + 1) * 512)
                        nc.tensor.matmul(
                            pss[(dt, c)][:], wkt[kd][:, dt * 128:(dt + 1) * 128],
                            keyt[kd][:, sl], start=(kd == 0), stop=(kd == 1))
            for dt in range(2):
                for c in range(4):
                    sl = slice(c * 512, (c + 1) * 512)
                    nc.vector.tensor_copy(kT[dt][:, sl], pss[(dt, c)][:])

        # ---- phase B: qsT = z^T @ (C/16)  [256, 1152]; 6 chains interleaved --
        qsT = [qkpool.tile([128, NQH], f32r, tag=f"qsT{i}", name=f"qsT{i}") for i in range(2)]
        with tc.tile_pool(name="psB", bufs=6, space="PSUM") as psB:
            pss = {}
            for dt in range(2):
                for c in range(3):
                    pss[(dt, c)] = psB.tile([128, 384], f32, tag="psB", name="psB")
            for kt in range(9):
                kr = 128 if kt < 8 else 2
                for dt in range(2):
                    for c in range(3):
                        sl = slice(c * 384, (c + 1) * 384)
                        nc.tensor.matmul(
                            pss[(dt, c)][:], zbuf[kt][:kr, dt * 128:(dt + 1) * 128],
                            cts[kt][:kr, sl], start=(kt == 0), stop=(kt == 8))
            for dt in range(2):
                for c in range(3):
                    sl = slice(c * 384, (c + 1) * 384)
                    nc.vector.tensor_copy(qsT[dt][:, sl], pss[(dt, c)][:])

        # ---- phase D: attention over 9 query tiles, software-pipelined ----
        with ExitStack() as jctx:
            psS = jctx.enter_context(tc.tile_pool(name="psS", bufs=2, space="PSUM"))
            psT = jctx.enter_context(tc.tile_pool(name="psT", bufs=2, space="PSUM"))
            psO = jctx.enter_context(tc.tile_pool(name="psO", bufs=2, space="PSUM"))
            work = jctx.enter_context(tc.tile_pool(name="work", bufs=3))
            ptp = jctx.enter_context(tc.tile_pool(name="ptp", bufs=4))
            stats = jctx.enter_context(tc.tile_pool(name="stats", bufs=4))

            state = {}  # per-jt carried tiles
            for step in range(NJT + 2):
                if step >= 2:
                    jt = step - 2
                    p_t, recip = state.pop(jt)
                    # 16 transposes packed 4-per-psum-bank, then one DVE copy
                    # per bank, then the 16 E accumulation steps (2 chains)
                    pt_sbs = []
                    for g in range(4):
                        pt_ps = psT.tile([128, 512], bf16, tag="psT", name="psT",
                                         padded_shape=[128, 1024])
                        for q in range(4):
                            kt = g * 4 + q
                            nc.tensor.matmul(pt_ps[:, q * 128:(q + 1) * 128],
                                             p_t[:, kt * 128:(kt + 1) * 128],
                                             id_b[:], is_transpose=True,
                                             start=True, stop=True)
                        pt_sb = ptp.tile([128, 512], bf16, tag="pt", name="pt")
                        nc.vector.tensor_copy(pt_sb[:], pt_ps[:])
                        pt_sbs.append(pt_sb)
                    po = [psO.tile([128, D], f32, tag="psO", name="psO",
                                   padded_shape=[128, 512])
                          for _ in range(2)]
                    for kt in range(NKT):
                        g, q = divmod(kt, 4)
                        nc.tensor.matmul(po[kt % 2][:],
                                         pt_sbs[g][:, q * 128:(q + 1) * 128],
                                         keyn[kt][:],
                                         start=(kt < 2), stop=(kt >= NKT - 2))
                    osb0 = work.tile([128, D], f32, tag="osb0", name="osb0")
                    nc.scalar.activation(out=osb0[:], in_=po[0][:],
                                         func=mybir.ActivationFunctionType.Copy,
                                         scale=recip[:])
                    osb = work.tile([128, D], f32, tag="osb", name="osb")
                    nc.vector.scalar_tensor_tensor(
                        out=osb[:], in0=po[1][:], scalar=recip[:], in1=osb0[:],
                        op0=mybir.AluOpType.mult, op1=mybir.AluOpType.add)
                    nc.sync.dma_start(dram["ob"][jt * 128:(jt + 1) * 128, :], osb[:])
                if step < NJT:
                    jt = step
                    jsl = slice(jt * 128, (jt + 1) * 128)
                    # scores in two 1024-wide halves (2 psum banks each);
                    # within a half the two 512-chunks interleave the K steps
                    halves = []
                    for h in range(2):
                        sh = psS.tile([128, 1024], f32, tag="psS", name="psS")
                        for dt in range(2):
                            for c in range(2):
                                sl = slice(c * 512, (c + 1) * 512)
                                ksl = slice(h * 1024 + c * 512, h * 1024 + (c + 1) * 512)
                                nc.tensor.matmul(
                                    sh[:, sl], qsT[dt][:, jsl], kT[dt][:, ksl],
                                    start=(dt == 0), stop=(dt == 1))
                        halves.append(sh)
                    mx = [stats.tile([128, 1], f32, tag=f"mx{h}", name=f"mx{h}") for h in range(2)]
                    for h in range(2):
                        nc.vector.reduce_max(out=mx[h][:], in_=halves[h][:],
                                             axis=mybir.AxisListType.X, negate=True)
                    negmax = stats.tile([128, 1], f32, tag="negmax", name="negmax")
                    nc.vector.tensor_scalar_min(negmax[:], mx[0][:], mx[1][:])
                    p_t = work.tile([128, NSEQ], bf16, tag="p", name="p")
                    sm = [stats.tile([128, 1], f32, tag=f"sm{h}", name=f"sm{h}") for h in range(2)]
                    for h in range(2):
                        nc.scalar.activation(
                            out=p_t[:, h * 1024:(h + 1) * 1024], in_=halves[h][:],
                            func=mybir.ActivationFunctionType.Exp,
                            bias=negmax[:], scale=1.0, accum_out=sm[h][:])
                    rsum = stats.tile([128, 1], f32, tag="rsum", name="rsum")
                    nc.vector.tensor_scalar_add(rsum[:], sm[0][:], sm[1][:])
                    recip = stats.tile([128, 1], f32, tag="recip", name="recip")
                    nc.vector.reciprocal(recip[:], rsum[:])
                    state[jt] = (p_t, recip)



def _host_prep(query, key, Wq, bq, Wk, bk):
    """Build per-core input maps (fold+transpose query, transpose key/weights,
    cosine table)."""
    query = np.ascontiguousarray(query, dtype=np.float32)
    key = np.ascontiguousarray(key, dtype=np.float32)

    nn = np.arange(NFOLD - 1, dtype=np.float64)          # 0..1024
    jj = np.arange(NQH, dtype=np.float64)
    ct = np.empty((NFOLD, NQH), dtype=np.float32)
    ct[:-1] = (np.cos(2.0 * np.pi * np.outer(nn, jj) / NSEQ) * SCALE).astype(np.float32)
    ct[-1] = SCALE  # bias row (ones * scale)
    cth = ct.astype(ml_dtypes.bfloat16)
    ctl = (ct - cth.astype(np.float32)).astype(ml_dtypes.bfloat16)

    wqt = np.ascontiguousarray(Wq.T, dtype=np.float32)
    wkt = np.ascontiguousarray(Wk.T, dtype=np.float32)
    bq2 = np.ascontiguousarray(bq.reshape(1, D), dtype=np.float32)

    in_maps = []
    for b in range(B):
        x = query[b]
        y = np.empty((1025, D), dtype=np.float32)
        y[0] = x[0]
        y[1:1024] = x[1:1024] + x[2047:1024:-1]
        y[1024] = x[1024]
        in_maps.append({
            "yt": np.ascontiguousarray(y.T),
            "bq": bq2,
            "wqt": wqt,
            "wkt": wkt,
            "keyt": np.ascontiguousarray(key[b].T),
            "keyn": np.ascontiguousarray(key[b]).astype(ml_dtypes.bfloat16),
            "cth": cth,
            "ctl": ctl,
            "ident": np.eye(128, dtype=ml_dtypes.bfloat16),
        })
    return in_maps


def kernel(query, key, Wq, bq, Wk, bk, _trace=False, _trace_kwargs=None):
    if "nc" not in _compiled:
        _compiled["nc"] = _build_module()
    nc = _compiled["nc"]

    in_maps = _host_prep(query, key, Wq, bq, Wk, bk)
    kw = {}
    if _trace:
        kw["trace"] = True
        if _trace_kwargs:
            kw.update(_trace_kwargs)
    res = run_bass_kernel_spmd(nc, in_maps, core_ids=list(range(B)), **kw)
    _compiled["last_results"] = res

    out = np.empty((B, NSEQ, D), dtype=np.float32)
    for b in range(B):
        ob = res.results[b]["ob"]
        out[b, :1025] = ob[:1025]
        out[b, 1025:] = ob[1023:0:-1]
    return out
